# revision 40
# baseline (speedup 1.0000x reference)
"""Trainium2 Bass kernel for a 6-layer transformer decoder (B=8, S=512, D=512,
H=8, DK=DV=64, DFF=2048, vocab 32000).

Strategy: data-parallel over the batch — each of the 8 NeuronCores runs the
full decoder stack for one batch element. No collectives.

v2 design notes (vs the v1 baseline):
- All matmuls in bf16 (FWL weight loads, full-rate at any free size).
- LayerNorm is *deferred through the next linear layer*: the gain is folded
  into the following weight matrix on the host, the raw residual feeds the
  matmuls immediately, and the per-token (x - mu) * rsd correction is applied
  as a rank-1 PSUM accumulation (colsum ⊗ -mu) plus a multiplicative rsd
  broadcast fused into the PSUM->SBUF drain. The PE therefore never waits on
  the LN statistic chain.
- LN stats: negated-mean and mean-square rows land in one PSUM bank via two
  sequential accumulation groups; rsd = Sqrt(reciprocal_approx(var)); rows are
  broadcast across partitions with gpsimd.partition_broadcast (no PSUM banks).
- Residual adds, LN applies and causal tri-masking run on the otherwise idle
  GpSimd engine (SBUF-only operands).
- Softmax: scores are computed into 2-bank PSUM tiles, exp'd in halved-pair
  ACT ops (uninitialized causal regions are exp'd harmlessly and never read),
  denominators come from the v-augmentation ones column, reciprocals are taken
  directly on the PSUM denominator row, and one K=2 selector matmul broadcasts
  both heads' reciprocal rows in one shot.
"""

import os
import numpy as np

_CONCOURSE_PATHS = ["/opt/trn_rl_repo", "/root/.axon_site/_ro/trn_rl_repo"]


def _ensure_path():
    try:
        import concourse.bass  # noqa: F401
    except Exception:
        import sys

        for p in _CONCOURSE_PATHS:
            if p not in sys.path and os.path.isdir(p):
                sys.path.insert(0, p)


V, D, NL, DK, DVh, H, DFF = 32000, 512, 6, 64, 64, 8, 2048
B, S = 8, 512
EPS = 1e-5
P = 128
NSUB = D // P  # 4 d-subtiles
NCH = S // P  # 4 s-chunks
NF = DFF // P  # 16 dff-chunks
HW_COLS = H * (DVh + 1)  # 520 augmented-v columns

# Debug knobs (test.py may override before calling kernel()).
N_LAYERS = NL
TAPS = ()
MM_DT = "bf16"  # v2 is bf16-only; knob kept for test.py compat

LAST_RESULT = None

_BUILD_CACHE = {}


def _pe_table():
    pos = np.arange(S)[:, None].astype(np.float32)
    i = np.arange(0, D, 2).astype(np.float32)
    ang = pos / np.power(10000.0, i / D)
    pe = np.zeros((S, D), dtype=np.float32)
    pe[:, 0::2] = np.sin(ang)
    pe[:, 1::2] = np.cos(ang)
    return pe


def _to_T_tiles(mat):
    """[S, D]-like -> [P, NSUB, S] transposed-tile layout (mat.T chunked)."""
    t = np.ascontiguousarray(np.asarray(mat, np.float32)).T  # [D, S]
    return np.ascontiguousarray(t.reshape(t.shape[0] // P, P, -1).transpose(1, 0, 2))


def _col_layout(vec):
    """[D]-like -> [P, D//P] per-partition column layout."""
    v = np.asarray(vec, np.float32).reshape(-1)
    return np.ascontiguousarray(v.reshape(v.shape[0] // P, P).T)


def _build(n_layers, causal_self, self_needs_mask, cross_needs_mask, flags, taps):
    _ensure_path()
    import concourse.mybir as mybir
    from concourse import bacc
    from concourse.tile import TileContext

    dt = mybir.dt
    AF = mybir.ActivationFunctionType
    OP = mybir.AluOpType
    f32 = dt.float32
    bf = dt.bfloat16

    (zb_qks0, zb_kvc, zb_qc, zb_qks, zb_w1c, zb_b2, ln_triv1, ln_triv2) = flags

    nc = bacc.Bacc("TRN2", target_bir_lowering=False, debug=False, num_devices=8)

    def din(name, shape, d=bf):
        return nc.dram_tensor(name, shape, d, kind="ExternalInput")

    x0T_d = din("x0T", [P, NSUB, S])
    encT_d = din("encT", [P, NSUB, S])
    ones_d = din("ones_row", [1, S])
    invD2_d = din("invD2", [P, 2])  # col0 = -1/D, col1 = +1/D
    sel2_d = din("sel2", [2, P])
    tri_d = din("tri01", [P, P]) if causal_self else None
    ident_d = din("ident", [P, P]) if (self_needs_mask or cross_needs_mask) else None
    smask_d = din("smaskT8", [P, NCH, S]) if self_needs_mask else None
    cmask_d = din("cmaskT8", [P, NCH, S]) if cross_needs_mask else None

    wq_s_d = din("wq_s", [n_layers, P, NSUB, D])
    wk_s_d = din("wk_s", [n_layers, P, NSUB, D])
    wv_s_d = din("wv_s", [n_layers, P, NSUB, HW_COLS])
    bv_s_d = din("bv_s", [n_layers, 1, HW_COLS])
    wq_c_d = din("wq_c", [n_layers, P, NSUB, D])
    wk_c_d = din("wk_c", [n_layers, P, NSUB, D])
    wv_c_d = din("wv_c", [n_layers, P, NSUB, HW_COLS])
    bv_c_d = din("bv_c", [n_layers, 1, HW_COLS])
    w1_d = din("w1", [n_layers, P, NSUB, DFF])
    w2_d = din("w2", [n_layers, P, NF, D])
    cs_qs_d = din("cs_qs", [n_layers, 1, D])
    cs_ks_d = din("cs_ks", [n_layers, 1, D])
    cs_qc_d = din("cs_qc", [n_layers, 1, D])
    cs_w1_d = din("cs_w1", [n_layers, 1, DFF])
    # optional bias / nontrivial-LN tensors
    bqk_s_row_d = None
    if not zb_qks0:
        bqk_s_row_d = (din("bq_s_row", [n_layers, 1, D]), din("bk_s_row", [n_layers, 1, D]))
    bkc_row_d = din("bk_c_row", [n_layers, 1, D]) if not zb_kvc else None
    bqc_col_d = din("bq_c_col", [n_layers, P, NSUB], f32) if not zb_qc else None
    bqs_col_d = din("bq_s_col", [n_layers, P, NSUB], f32) if not zb_qks else None
    bks_col_d = din("bk_s_col", [n_layers, P, NSUB], f32) if not zb_qks else None
    cw1_d = din("c_w1", [n_layers, 1, DFF]) if not zb_w1c else None
    b2c_d = din("b2c", [n_layers, P, NSUB], f32) if not zb_b2 else None
    g1c_d = b1c_ln_d = None
    if not ln_triv1:
        g1c_d = din("ln1g", [n_layers, P, NSUB], f32)
        b1c_ln_d = din("ln1b", [n_layers, P, NSUB], f32)
    g2c_d = b2c_ln_d = None
    if not ln_triv2:
        g2c_d = din("ln2g", [n_layers, P, NSUB], f32)
        b2c_ln_d = din("ln2b", [n_layers, P, NSUB], f32)

    out_d = nc.dram_tensor("out_xT", [P, NSUB, S], f32, kind="ExternalOutput")
    tap_d = {
        t: nc.dram_tensor(f"tap_{t}", [P, NSUB, S], bf, kind="ExternalOutput")
        for t in taps
    }

    def mm(out, lhsT, rhs, start, stop):
        nc.tensor.matmul(out, lhsT, rhs, start=start, stop=stop, skip_group_check=True)

    with TileContext(nc) as tc:
        with (
            nc.allow_low_precision(reason="bf16 matmul pipeline"),
            tc.tile_pool(name="const", bufs=1) as cpool,
            tc.tile_pool(name="w4", bufs=7) as w4pool,
            tc.tile_pool(name="w8", bufs=3) as w8pool,
            tc.tile_pool(name="x", bufs=5) as xpool,
            tc.tile_pool(name="xo", bufs=1) as xopool,
            tc.tile_pool(name="qk", bufs=4) as qkpool,
            tc.tile_pool(name="v", bufs=2) as vpool,
            tc.tile_pool(name="exp", bufs=3) as epool,
            tc.tile_pool(name="attn", bufs=3) as apool,
            tc.tile_pool(name="sq", bufs=2) as sqpool,
            tc.tile_pool(name="ff", bufs=1) as ffpool,
            tc.tile_pool(name="bc", bufs=2) as bcpool,
            tc.tile_pool(name="row", bufs=2) as rpool,
            tc.tile_pool(name="sm", bufs=2) as spool,
            tc.tile_pool(name="ps", bufs=1, space="PSUM") as pspool,
        ):
            # ---- constants ----
            ones_sb = cpool.tile([1, S], bf, tag="c_ones")
            nc.sync.dma_start(ones_sb[:], ones_d[:])
            invD2_sb = cpool.tile([P, 2], bf, tag="c_invD2")
            nc.sync.dma_start(invD2_sb[:], invD2_d[:])
            sel2_sb = cpool.tile([2, P], bf, tag="c_sel2")
            nc.sync.dma_start(sel2_sb[:], sel2_d[:])
            if causal_self:
                tri_sb = cpool.tile([P, P], bf, tag="c_tri")
                nc.sync.dma_start(tri_sb[:], tri_d[:])
            id_sb = None
            if ident_d is not None:
                id_sb = cpool.tile([P, P], bf, tag="c_id")
                nc.sync.dma_start(id_sb[:], ident_d[:])
            smask_sb = None
            if self_needs_mask:
                smask_sb = cpool.tile([P, NCH, S], bf, tag="c_smask")
                nc.sync.dma_start(smask_sb[:], smask_d[:])
            cmask_sb = None
            if cross_needs_mask:
                cmask_sb = cpool.tile([P, NCH, S], bf, tag="c_cmask")
                nc.sync.dma_start(cmask_sb[:], cmask_d[:])
            encT = cpool.tile([P, NSUB, S], bf, tag="c_enc")
            nc.sync.dma_start(encT[:], encT_d[:])
            xT = cpool.tile([P, NSUB, S], bf, tag="c_x0")
            nc.sync.dma_start(xT[:], x0T_d[:])

            _n = [0]

            def ps2():
                _n[0] += 1
                return pspool.tile([P, 2, S], f32, tag="ps2", bufs=3, name=f"p2_{_n[0]}")

            def ps1():
                _n[0] += 1
                return pspool.tile([P, S], f32, tag="ps1", bufs=2, name=f"p1_{_n[0]}")

            def psb():
                # single-bank bf16 score tile (pair of 512-wide c-chunks)
                _n[0] += 1
                return pspool.tile([P, 2, S], bf, tag="psb", bufs=3, name=f"pb_{_n[0]}")

            def load_w4(src, l):
                t = w4pool.tile([P, NSUB, HW_COLS], bf, tag="wt4")
                nc.sync.dma_start(t[:, :, : src.shape[3]], src[l])
                return t

            def load_row(src, l, n, tag=None):
                t = spool.tile(
                    [1, n], bf, tag=tag or f"row{n}",
                    bufs=3 if n <= HW_COLS else 1, name="lrow",
                )
                nc.sync.dma_start(t[0:1, 0:n], src[l])
                return t

            def load_col(src, l, tag="bcol"):
                t = spool.tile([P, NSUB], f32, tag=tag)
                nc.sync.dma_start(t[:], src[l])
                return t

            def tap(name, tile_):
                if name in tap_d:
                    nc.sync.dma_start(tap_d[name][:], tile_[:])

            # ---------- building blocks ----------

            def proj_raw2(w_sb, srcT, brow=None):
                """16 accumulation MMs into two 2-bank PSUM tiles; returns the
                undrained PSUM tiles."""
                out = []
                for t in range(2):
                    ps = ps2()
                    for g in range(2):
                        j = 2 * t + g
                        last = NSUB - 1
                        for i in range(NSUB):
                            mm(
                                ps[:, g, :],
                                w_sb[:, i, j * P : (j + 1) * P],
                                srcT[:, i, :],
                                start=(i == 0),
                                stop=(brow is None and i == last),
                            )
                        if brow is not None:
                            mm(
                                ps[:, g, :],
                                ones_sb[0:1, 0:P],
                                brow[0:1, j * P : (j + 1) * P],
                                start=False,
                                stop=True,
                            )
                    out.append(ps)
                return out

            def proj_half(w_sb, srcT, t, brow=None):
                """one 2-bank PSUM tile covering output subtiles 2t, 2t+1"""
                ps = ps2()
                for g in range(2):
                    j = 2 * t + g
                    for i in range(NSUB):
                        mm(
                            ps[:, g, :],
                            w_sb[:, i, j * P : (j + 1) * P],
                            srcT[:, i, :],
                            start=(i == 0),
                            stop=(brow is None and i == NSUB - 1),
                        )
                    if brow is not None:
                        mm(
                            ps[:, g, :],
                            ones_sb[0:1, 0:P],
                            brow[0:1, j * P : (j + 1) * P],
                            start=False,
                            stop=True,
                        )
                return ps

            def outer_half(ps, t, cs_row, nm_row):
                for g in range(2):
                    j = 2 * t + g
                    mm(
                        ps[:, g, :],
                        cs_row[0:1, j * P : (j + 1) * P],
                        nm_row[0:1, :],
                        start=False,
                        stop=True,
                    )

            def drain_mult_half(ps, t, out_t, rsd_bc, bcol=None):
                nc.vector.tensor_tensor(
                    out_t[:, 2 * t : 2 * t + 2, :],
                    ps[:, :, :],
                    rsd_bc.broadcast_to([P, 2, S]),
                    OP.mult,
                )
                if bcol is not None:
                    for g in range(2):
                        j = 2 * t + g
                        nc.vector.tensor_scalar(
                            out_t[:, j, :], out_t[:, j, :],
                            bcol[:, j : j + 1], None, OP.add,
                        )

            def proj_outer(ps_tiles, cs_row, nm_row):
                """rank-1 correction: ps[j] += colsum_j ⊗ negmean."""
                for t, ps in enumerate(ps_tiles):
                    for g in range(2):
                        j = 2 * t + g
                        mm(
                            ps[:, g, :],
                            cs_row[0:1, j * P : (j + 1) * P],
                            nm_row[0:1, :],
                            start=False,
                            stop=True,
                        )

            def proj_drain_mult(ps_tiles, out_t, rsd_bc, bcol=None):
                """out[:, j, :] = ps[j] * rsd  (deferred-LN drain)."""
                for t, ps in enumerate(ps_tiles):
                    nc.vector.tensor_tensor(
                        out_t[:, 2 * t : 2 * t + 2, :],
                        ps[:, :, :],
                        rsd_bc.broadcast_to([P, 2, S]),
                        OP.mult,
                    )
                    if bcol is not None:
                        for g in range(2):
                            j = 2 * t + g
                            nc.vector.tensor_scalar(
                                out_t[:, j, :], out_t[:, j, :],
                                bcol[:, j : j + 1], None, OP.add,
                            )

            def proj_drain_copy(ps_tiles, out_t, engine_alt=False):
                for t, ps in enumerate(ps_tiles):
                    if engine_alt and t == 0:
                        nc.scalar.activation(
                            out_t[:, 2 * t : 2 * t + 2, :], ps[:, :, :], AF.Copy
                        )
                    else:
                        nc.vector.tensor_copy(
                            out_t[:, 2 * t : 2 * t + 2, :], ps[:, :, :]
                        )

            def v_aug(w_sb, brow_sb, srcT):
                """augmented v, natural orientation: [P(s), NCH, 520]."""
                vt = vpool.tile([P, NCH, HW_COLS], bf, tag="v")
                half = HW_COLS // 2  # 260
                for sc in range(NCH):
                    ps = ps2()
                    for hh in range(2):
                        cs, ce = hh * half, (hh + 1) * half
                        for i in range(NSUB):
                            mm(
                                ps[:, hh, 0:half],
                                srcT[:, i, sc * P : (sc + 1) * P],
                                w_sb[:, i, cs:ce],
                                start=(i == 0),
                                stop=False,
                            )
                        mm(
                            ps[:, hh, 0:half],
                            ones_sb[0:1, 0:P],
                            brow_sb[0:1, cs:ce],
                            start=False,
                            stop=True,
                        )
                    nc.vector.tensor_copy(
                        vt[:, sc, :].rearrange("p (h c) -> p h c", h=2),
                        ps[:, :, 0:half],
                    )
                return vt

            def attention(qT, kT, vt, attnT, causal, mask_sb):
                def flush_rb(rc2, j):
                    # K=1 broadcasts of the reciprocal rows, then in-place
                    # normalize; emitted one pair late so the PE never waits
                    rb = ps1()
                    for u in range(2):
                        mm(
                            rb[u * 64 : u * 64 + 64, :],
                            ones_sb[0:1, 0:64],
                            rc2[0:1, u, :],
                            start=True,
                            stop=True,
                        )
                    for u in range(2):
                        nc.vector.tensor_tensor(
                            attnT[u * 64 : u * 64 + 64, j, :],
                            attnT[u * 64 : u * 64 + 64, j, :],
                            rb[u * 64 : u * 64 + 64, :],
                            OP.mult,
                        )

                pend = None
                for j in range(NSUB):
                    exs = [
                        epool.tile([P, NCH, S], bf, tag="exp", name=f"ex{j}_0"),
                        epool.tile([P, NCH, S], bf, tag="exp", name=f"ex{j}_1"),
                    ]
                    avs = ps2()
                    done_sc = []
                    for h in range(2):
                        scs = [ps2(), ps2()]
                        for u in range(2):
                            for ci in range(2):
                                c = 2 * h + ci
                                qs = c * P if causal else 0
                                mm(
                                    scs[u][:, ci, qs:S],
                                    kT[u * 64 : u * 64 + 64, j, c * P : (c + 1) * P],
                                    qT[u * 64 : u * 64 + 64, j, qs:S],
                                    start=True,
                                    stop=(mask_sb is None),
                                )
                                if mask_sb is not None:
                                    mm(
                                        scs[u][:, ci, qs:S],
                                        id_sb[:],
                                        mask_sb[:, c, qs:S],
                                        start=False,
                                        stop=True,
                                    )
                        for ci in range(2):
                            c = 2 * h + ci
                            q0 = c * P if causal else 0
                            for u in range(2):
                                nc.scalar.activation(
                                    exs[u][:, c, q0:S],
                                    scs[u][:, ci, q0:S],
                                    AF.Exp,
                                    scale=0.125,
                                )
                                if causal:
                                    nc.gpsimd.tensor_tensor(
                                        exs[u][:, c, c * P : (c + 1) * P],
                                        exs[u][:, c, c * P : (c + 1) * P],
                                        tri_sb[:],
                                        OP.mult,
                                    )
                        done_sc.append(scs)
                    for u in range(2):
                        hh = 2 * j + u
                        for c in range(NCH):
                            q0 = c * P if causal else 0
                            mm(
                                avs[0:65, u, q0:S],
                                vt[:, c, hh * 65 : (hh + 1) * 65],
                                exs[u][:, c, q0:S],
                                start=(c == 0),
                                stop=(c == NCH - 1),
                            )
                    if pend is not None:
                        flush_rb(*pend)
                    # drain the raw AV immediately (frees the PSUM tile
                    # without waiting for the reciprocal chain)...
                    for u in range(2):
                        nc.vector.tensor_copy(
                            attnT[u * 64 : u * 64 + 64, j, :], avs[0:64, u, :]
                        )
                    rs2 = rpool.tile([1, 2, S], f32, tag="rs2", name=f"rs2{j}")
                    nc.scalar.activation(rs2[0:1, :, :], avs[64:65, :, :], AF.Copy)
                    # ...then normalize in place once the reciprocal lands
                    rcf = rpool.tile([1, 2, S], f32, tag="rcf", bufs=1, name=f"rcf{j}")
                    nc.vector.reciprocal_approx_fast(rcf[0:1, :, :], rs2[0:1, :, :])
                    rc2 = rpool.tile([1, 2, S], bf, tag="rc2", name=f"rc2{j}")
                    nc.vector.tensor_copy(rc2[0:1, :, :], rcf[0:1, :, :])
                    pend = (rc2, j)
                flush_rb(*pend)

            def residual(a_T, b_T, name):
                """per-subtile so downstream matmuls start as heads finish"""
                xo = xpool.tile([P, NSUB, S], bf, tag="x", name=name)
                for i in range(NSUB):
                    nc.vector.tensor_tensor(
                        xo[:, i, :], a_T[:, i, :], b_T[:, i, :], OP.add
                    )
                return xo

            def ln_start(r_T):
                """squares on ACT only — PE stats MMs go in ln_finish so
                covering matmuls can sit between them in the PE queue"""
                sq = sqpool.tile([P, NSUB, S], bf, tag="sq")
                nc.scalar.activation(sq[:], r_T[:], AF.Square)
                return sq

            def ln_finish(r_T, sq):
                st = ps1()
                for i in range(NSUB):
                    mm(st[0:1, :], invD2_sb[:, 0:1], r_T[:, i, :],
                       start=(i == 0), stop=(i == NSUB - 1))
                for i in range(NSUB):
                    mm(st[32:33, :], invD2_sb[:, 1:2], sq[:, i, :],
                       start=(i == 0), stop=(i == NSUB - 1))
                rows = rpool.tile([1, 3, S], f32, tag="lnrow", bufs=1, name="lnrows")
                nm_bf = rpool.tile([1, S], bf, tag="lnnm", name="lnnm")
                rsd_bf = rpool.tile([1, S], bf, tag="lnrsd", name="lnrsd")
                # rows segments: 0=msq, 1=var, 2=rvar
                nc.scalar.activation(rows[0:1, 0, :], st[0:1, :], AF.Square)
                nc.scalar.activation(nm_bf[0:1, :], st[0:1, :], AF.Copy)
                nc.vector.scalar_tensor_tensor(
                    rows[0:1, 1, :], st[32:33, :], float(EPS), rows[0:1, 0, :],
                    OP.add, OP.subtract,
                )
                nc.vector.reciprocal_approx_fast(rows[0:1, 2, :], rows[0:1, 1, :])
                nc.scalar.activation(rsd_bf[0:1, :], rows[0:1, 2, :], AF.Sqrt)
                bc = bcpool.tile([P, 2, S], bf, tag="bc")
                nc.gpsimd.partition_broadcast(bc[:, 0, :], nm_bf[0:1, :])
                nc.gpsimd.partition_broadcast(bc[:, 1, :], rsd_bf[0:1, :])
                return nm_bf, bc, rows

            def ln_apply(r_T, bc, gcol, bcol, out_f32=False, name="xa"):
                tmp = sqpool.tile([P, NSUB, S], bf, tag="sq", name=f"t{name}")
                nc.vector.tensor_tensor(
                    tmp[:], r_T[:], bc[:, 0:1, :].broadcast_to([P, NSUB, S]), OP.add
                )
                xo = (xopool if out_f32 else xpool).tile(
                    [P, NSUB, S], f32 if out_f32 else bf,
                    tag="xo" if out_f32 else "x", name=name,
                )
                nc.vector.tensor_tensor(
                    xo[:], tmp[:], bc[:, 1:2, :].broadcast_to([P, NSUB, S]), OP.mult
                )
                if gcol is not None:
                    for i in range(NSUB):
                        nc.scalar.activation(
                            xo[:, i, :], xo[:, i, :], AF.Identity,
                            bias=bcol[:, i : i + 1], scale=gcol[:, i : i + 1],
                        )
                return xo

            # ---------- the decoder stack ----------

            qsT = qkpool.tile([P, NSUB, S], bf, tag="qk", name="qs0")
            ksT = qkpool.tile([P, NSUB, S], bf, tag="qk", name="ks0")
            wq0 = load_w4(wq_s_d, 0)
            wk0 = load_w4(wk_s_d, 0)
            brow_q0 = brow_k0 = None
            if not zb_qks0:
                brow_q0 = load_row(bqk_s_row_d[0], 0, D, tag="brow")
                brow_k0 = load_row(bqk_s_row_d[1], 0, D, tag="brow")
            q_ps = proj_raw2(wq0, xT, brow_q0)
            k_ps = proj_raw2(wk0, xT, brow_k0)
            proj_drain_copy(q_ps, qsT)
            proj_drain_copy(k_ps, ksT, engine_alt=True)

            for l in range(n_layers):
                # ---- self attention ----
                wv = load_w4(wv_s_d, l)
                bv = load_row(bv_s_d, l, HW_COLS, tag="brow")
                vt = v_aug(wv, bv, xT)
                saT = apool.tile([P, NSUB, S], bf, tag="attn", name=f"sa{l}")
                attention(qsT, ksT, vt, saT, causal_self, smask_sb)
                tap(f"sa{l}", saT)

                r1 = residual(xT, saT, f"r1_{l}")
                sq1 = ln_start(r1)

                # cover the residual + LN1 chain latency: cross k/v first
                # (independent of r1), then cross-q raw, then the stats MMs
                wkc = load_w4(wk_c_d, l)
                brow_kc = load_row(bkc_row_d, l, D, tag="brow") if not zb_kvc else None
                kc_ps = proj_raw2(wkc, encT, brow_kc)
                kcT = qkpool.tile([P, NSUB, S], bf, tag="qk", name=f"kc{l}")
                proj_drain_copy(kc_ps, kcT, engine_alt=True)
                wvc = load_w4(wv_c_d, l)
                bvc = load_row(bv_c_d, l, HW_COLS, tag="brow")
                vc = v_aug(wvc, bvc, encT)
                wqc = load_w4(wq_c_d, l)
                cs_qc = load_row(cs_qc_d, l, D)
                qc_ps = proj_raw2(wqc, r1, None)
                nm1, bc1, _rows1 = ln_finish(r1, sq1)
                proj_outer(qc_ps, cs_qc, nm1[0:1, :])
                qcT = qkpool.tile([P, NSUB, S], bf, tag="qk", name=f"qc{l}")
                proj_drain_mult(
                    qc_ps, qcT, bc1[:, 1:2, :],
                    bcol=load_col(bqc_col_d, l) if not zb_qc else None,
                )
                x1 = ln_apply(
                    r1, bc1,
                    None if ln_triv1 else load_col(g1c_d, l, tag="gcol"),
                    None if ln_triv1 else load_col(b1c_ln_d, l, tag="gcol"),
                    name=f"x1_{l}",
                )
                tap(f"x1_{l}", x1)

                # ---- cross attention ----
                caT = apool.tile([P, NSUB, S], bf, tag="attn", name=f"ca{l}")
                attention(qcT, kcT, vc, caT, False, cmask_sb)
                tap(f"ca{l}", caT)

                r2 = residual(x1, caT, f"r2_{l}")
                sq2 = ln_start(r2)

                # ---- FFN (LN2 deferred through W1, rsd deferred through W2) ----
                cs_w1 = load_row(cs_w1_d, l, DFF)
                w1g = [None, None]
                for g in range(2):
                    w1g[g] = w8pool.tile([P, NSUB, 1024], bf, tag="wt8", name=f"w1g{g}")
                    nc.sync.dma_start(
                        w1g[g][:], w1_d[l, :, :, g * 1024 : (g + 1) * 1024]
                    )
                ff1 = ffpool.tile([P, NF, S], bf, tag="ff1")
                ffps = {}
                nm2 = bc2 = rows2 = cvec = sd_row = None

                def w1_raw(t):
                    ps = ps2()
                    for g in range(2):
                        F = 2 * t + g
                        gr, off = divmod(F * P, 1024)
                        for i in range(NSUB):
                            mm(
                                ps[:, g, :],
                                w1g[gr][:, i, off : off + P],
                                r2[:, i, :],
                                start=(i == 0),
                                stop=False,
                            )
                    ffps[t] = ps

                def w1_fin(t):
                    ps = ffps[t]
                    for g in range(2):
                        F = 2 * t + g
                        mm(
                            ps[:, g, :],
                            cs_w1[0:1, F * P : (F + 1) * P],
                            nm2[0:1, :],
                            start=False,
                            stop=(cvec is None),
                        )
                        if cvec is not None:
                            mm(
                                ps[:, g, :],
                                cvec[0:1, F * P : (F + 1) * P],
                                sd_row[0:1, :],
                                start=False,
                                stop=True,
                            )
                    # drain with relu, alternating engines
                    if t % 2 == 0:
                        nc.vector.tensor_scalar(
                            ff1[:, 2 * t : 2 * t + 2, :], ps[:, :, :], 0.0, None, OP.max
                        )
                    else:
                        nc.scalar.activation(
                            ff1[:, 2 * t : 2 * t + 2, :], ps[:, :, :], AF.Relu
                        )

                w1_raw(0)
                nm2, bc2, rows2 = ln_finish(r2, sq2)
                if not zb_w1c:
                    cvec = load_row(cw1_d, l, DFF, tag="cw1row")
                    sd_row = rpool.tile([1, S], bf, tag="sdrow", name="sd")
                    nc.scalar.activation(sd_row[0:1, :], rows2[0:1, 1, :], AF.Sqrt)
                w1_raw(1)
                for t in range(8):
                    w1_fin(t)
                    if t + 2 < 8:
                        w1_raw(t + 2)

                x2 = ln_apply(
                    r2, bc2,
                    None if ln_triv2 else load_col(g2c_d, l, tag="gcol"),
                    None if ln_triv2 else load_col(b2c_ln_d, l, tag="gcol"),
                    name=f"x2_{l}",
                )
                tap(f"x2_{l}", x2)

                w2g = [None, None]
                for t in range(2):
                    w2g[t] = w8pool.tile([P, NF, 2 * P], bf, tag="wt8", name=f"w2g{t}")
                    nc.sync.dma_start(
                        w2g[t][:], w2_d[l, :, :, 2 * t * P : (2 * t + 2) * P]
                    )
                ffo = apool.tile([P, NSUB, S], bf, tag="attn", name=f"ffo{l}")
                for t in range(2):
                    ps = ps2()
                    for g in range(2):
                        for k in range(NF):
                            mm(
                                ps[:, g, :],
                                w2g[t][:, k, g * P : (g + 1) * P],
                                ff1[:, k, :],
                                start=(k == 0),
                                stop=(k == NF - 1),
                            )
                    nc.vector.tensor_tensor(
                        ffo[:, 2 * t : 2 * t + 2, :],
                        ps[:, :, :],
                        bc2[:, 1:2, :].broadcast_to([P, 2, S]),
                        OP.mult,
                    )
                    if not zb_b2:
                        b2col = load_col(b2c_d, l)
                        for g in range(2):
                            j = 2 * t + g
                            nc.vector.tensor_scalar(
                                ffo[:, j, :], ffo[:, j, :],
                                b2col[:, j : j + 1], None, OP.add,
                            )
                tap(f"ff{l}", ffo)

                r3 = residual(x2, ffo, f"r3_{l}")
                sq3 = ln_start(r3)

                last = l == n_layers - 1
                if not last:
                    wqn = load_w4(wq_s_d, l + 1)
                    wkn = load_w4(wk_s_d, l + 1)
                    cs_qs = load_row(cs_qs_d, l + 1, D)
                    cs_ks = load_row(cs_ks_d, l + 1, D)
                    bq_col = load_col(bqs_col_d, l + 1) if not zb_qks else None
                    bk_col = load_col(bks_col_d, l + 1) if not zb_qks else None
                    qsT = qkpool.tile([P, NSUB, S], bf, tag="qk", name=f"qs{l+1}")
                    ksT = qkpool.tile([P, NSUB, S], bf, tag="qk", name=f"ks{l+1}")
                    qn0 = proj_half(wqn, r3, 0)
                    kn0 = proj_half(wkn, r3, 0)
                    nm3, bc3, _rows3 = ln_finish(r3, sq3)
                    qn1 = proj_half(wqn, r3, 1)
                    outer_half(qn0, 0, cs_qs, nm3)
                    outer_half(kn0, 0, cs_ks, nm3)
                    outer_half(qn1, 1, cs_qs, nm3)
                    drain_mult_half(qn0, 0, qsT, bc3[:, 1:2, :], bq_col)
                    drain_mult_half(kn0, 0, ksT, bc3[:, 1:2, :], bk_col)
                    kn1 = proj_half(wkn, r3, 1)
                    outer_half(kn1, 1, cs_ks, nm3)
                    drain_mult_half(qn1, 1, qsT, bc3[:, 1:2, :], bq_col)
                    drain_mult_half(kn1, 1, ksT, bc3[:, 1:2, :], bk_col)
                else:
                    nm3, bc3, _rows3 = ln_finish(r3, sq3)
                xT = ln_apply(
                    r3, bc3,
                    None if ln_triv2 else load_col(g2c_d, l, tag="gcol"),
                    None if ln_triv2 else load_col(b2c_ln_d, l, tag="gcol"),
                    out_f32=last,
                    name=f"x3_{l}",
                )

            nc.sync.dma_start(out_d[:], xT[:])

    nc.compile()
    return nc


def _prep_shared(inputs, n_layers):
    """Host-side marshalling into device tile layouts (float32 arrays; cast
    to bf16 in kernel())."""
    g = {}
    emb = np.asarray(inputs["emb"], np.float32)
    ln1_g = np.asarray(inputs["ln1_g"], np.float32)
    ln2_g = np.asarray(inputs["ln2_g"], np.float32)

    def wqk_prep(w, gvec):  # [H,D,DK] (+fold g over D) -> [P, NSUB, D], colsum row
        w2d = np.asarray(w, np.float32).transpose(1, 0, 2).reshape(D, H * DK)
        if gvec is not None:
            w2d = w2d * gvec[:, None]
        cs = w2d.sum(axis=0)  # [H*DK]
        return (
            np.ascontiguousarray(w2d.reshape(NSUB, P, H * DK).transpose(1, 0, 2)),
            cs.reshape(1, -1),
        )

    def wv_prep(w, bv):
        aug = np.zeros((D, HW_COLS), np.float32)
        baug = np.zeros(HW_COLS, np.float32)
        wl = np.asarray(w, np.float32)
        bl = np.asarray(bv, np.float32)
        for h in range(H):
            aug[:, h * 65 : h * 65 + 64] = wl[h]
            baug[h * 65 : h * 65 + 64] = bl[h]
            baug[h * 65 + 64] = 1.0
        return (
            np.ascontiguousarray(aug.reshape(NSUB, P, HW_COLS).transpose(1, 0, 2)),
            baug.reshape(1, -1),
        )

    wq_s = np.empty((n_layers, P, NSUB, D), np.float32)
    wk_s = np.empty((n_layers, P, NSUB, D), np.float32)
    wv_s = np.empty((n_layers, P, NSUB, HW_COLS), np.float32)
    bv_s = np.empty((n_layers, 1, HW_COLS), np.float32)
    wq_c = np.empty((n_layers, P, NSUB, D), np.float32)
    wk_c = np.empty((n_layers, P, NSUB, D), np.float32)
    wv_c = np.empty((n_layers, P, NSUB, HW_COLS), np.float32)
    bv_c = np.empty((n_layers, 1, HW_COLS), np.float32)
    cs_qs = np.zeros((n_layers, 1, D), np.float32)
    cs_ks = np.zeros((n_layers, 1, D), np.float32)
    cs_qc = np.zeros((n_layers, 1, D), np.float32)
    w1 = np.empty((n_layers, P, NSUB, DFF), np.float32)
    w2 = np.empty((n_layers, P, NF, D), np.float32)
    cs_w1 = np.zeros((n_layers, 1, DFF), np.float32)

    for l in range(n_layers):
        g3 = ln2_g[l - 1] if l >= 1 else None  # LN3 of prev layer gates self q/k
        wq_s[l], cs_qs[l] = wqk_prep(inputs["Wq_s"][l], g3)
        wk_s[l], cs_ks[l] = wqk_prep(inputs["Wk_s"][l], g3)
        wv_s[l], bv_s[l] = wv_prep(inputs["Wv_s"][l], inputs["bv_s"][l])
        wq_c[l], cs_qc[l] = wqk_prep(inputs["Wq_c"][l], ln1_g[l])
        wk_c[l], _ = wqk_prep(inputs["Wk_c"][l], None)
        wv_c[l], bv_c[l] = wv_prep(inputs["Wv_c"][l], inputs["bv_c"][l])
        w1l = np.asarray(inputs["W1"][l], np.float32) * ln2_g[l][:, None]
        cs_w1[l] = w1l.sum(axis=0).reshape(1, -1)
        w1[l] = w1l.reshape(NSUB, P, DFF).transpose(1, 0, 2)
        w2[l] = (
            np.asarray(inputs["W2"][l], np.float32)
            .reshape(NF, P, D)
            .transpose(1, 0, 2)
        )

    g.update(
        wq_s=wq_s, wk_s=wk_s, wv_s=wv_s, bv_s=bv_s,
        wq_c=wq_c, wk_c=wk_c, wv_c=wv_c, bv_c=bv_c,
        cs_qs=cs_qs, cs_ks=cs_ks, cs_qc=cs_qc, w1=w1, w2=w2, cs_w1=cs_w1,
    )
    g["ones_row"] = np.ones((1, S), np.float32)
    invD2 = np.zeros((P, 2), np.float32)
    invD2[:, 0] = -1.0 / D
    invD2[:, 1] = 1.0 / D
    g["invD2"] = invD2
    sel2 = np.zeros((2, P), np.float32)
    sel2[0, 0:64] = 1.0
    sel2[1, 64:128] = 1.0
    g["sel2"] = sel2
    q = np.arange(P)
    g["tri01"] = (q[None, :] >= q[:, None]).astype(np.float32)
    g["ident"] = np.eye(P, dtype=np.float32)
    return g, emb


def _mask_T8(mask_b):
    """[S, S] additive mask -> [P, NCH, S] transposed, pre-scaled by 8."""
    m = np.ascontiguousarray(np.asarray(mask_b, np.float32).T) * 8.0
    return np.ascontiguousarray(m.reshape(NCH, P, S).transpose(1, 0, 2))


def kernel(**inputs):
    global LAST_RESULT
    _ensure_path()
    import ml_dtypes
    from concourse.bass_utils import run_bass_kernel_spmd

    bf_np = ml_dtypes.bfloat16
    n_layers = N_LAYERS
    ids = np.asarray(inputs["decoder_input"])
    enc = np.asarray(inputs["encoder_output"], np.float32)
    smask = np.asarray(inputs["self_mask"], np.float32)
    cmask = np.asarray(inputs["cross_mask"], np.float32)

    tril = np.tril(np.ones((S, S), bool))
    canon = np.where(tril, np.float32(0.0), np.float32(-1e9))
    causal_self = all(np.array_equal(smask[b], canon) for b in range(B))
    self_needs_mask = (not causal_self) and bool(np.any(smask != 0.0))
    cross_needs_mask = bool(np.any(cmask != 0.0))

    def allz(key):
        return not bool(np.any(np.asarray(inputs[key])[:n_layers]))

    ln1_g = np.asarray(inputs["ln1_g"], np.float32)[:n_layers]
    ln2_g = np.asarray(inputs["ln2_g"], np.float32)[:n_layers]
    ln_triv1 = bool(np.all(ln1_g == 1.0)) and allz("ln1_b")
    ln_triv2 = bool(np.all(ln2_g == 1.0)) and allz("ln2_b")
    zb_qks0 = allz("bq_s") and allz("bk_s")
    zb_kvc = allz("bk_c")
    zb_qc = allz("bq_c") and allz("ln1_b")
    zb_qks = allz("bq_s") and allz("bk_s") and allz("ln2_b")
    zb_w1c = allz("b1") and allz("ln2_b")
    zb_b2 = allz("b2")
    flags = (zb_qks0, zb_kvc, zb_qc, zb_qks, zb_w1c, zb_b2, ln_triv1, ln_triv2)

    shared, emb = _prep_shared(inputs, n_layers)
    if not zb_qks0:
        bq = np.asarray(inputs["bq_s"], np.float32)[:n_layers]
        bk = np.asarray(inputs["bk_s"], np.float32)[:n_layers]
        shared["bq_s_row"] = bq.reshape(n_layers, 1, -1)
        shared["bk_s_row"] = bk.reshape(n_layers, 1, -1)
    if not zb_kvc:
        shared["bk_c_row"] = np.asarray(inputs["bk_c"], np.float32)[
            :n_layers
        ].reshape(n_layers, 1, -1)
    # nontrivial bias columns (rare path): bterm = W^T ln_b + b
    if not (zb_qc and zb_qks and zb_w1c and zb_b2 and ln_triv1 and ln_triv2):
        ln1_b = np.asarray(inputs["ln1_b"], np.float32)
        ln2_b = np.asarray(inputs["ln2_b"], np.float32)

        def bterm(wkey, bkey, lnb, l):
            w2d = (
                np.asarray(inputs[wkey][l], np.float32)
                .transpose(1, 0, 2)
                .reshape(D, H * DK)
            )
            return w2d.T @ lnb[l] + np.asarray(inputs[bkey][l], np.float32).reshape(-1)

        if not zb_qc:
            shared["bq_c_col"] = np.stack(
                [_col_layout(bterm("Wq_c", "bq_c", ln1_b, l)) for l in range(n_layers)]
            )
        if not zb_qks:
            shared["bq_s_col"] = np.stack(
                [
                    _col_layout(
                        bterm("Wq_s", "bq_s", np.roll(ln2_b, 1, 0), l)
                        if l >= 1
                        else np.zeros(D, np.float32)
                    )
                    for l in range(n_layers)
                ]
            )
            shared["bk_s_col"] = np.stack(
                [
                    _col_layout(
                        bterm("Wk_s", "bk_s", np.roll(ln2_b, 1, 0), l)
                        if l >= 1
                        else np.zeros(D, np.float32)
                    )
                    for l in range(n_layers)
                ]
            )
        if not zb_w1c:
            shared["c_w1"] = np.stack(
                [
                    (
                        np.asarray(inputs["W1"][l], np.float32).T @ ln2_b[l]
                        + np.asarray(inputs["b1"][l], np.float32)
                    ).reshape(1, -1)
                    for l in range(n_layers)
                ]
            )
        if not zb_b2:
            shared["b2c"] = np.stack(
                [_col_layout(np.asarray(inputs["b2"][l])) for l in range(n_layers)]
            )
        if not ln_triv1:
            shared["ln1g"] = np.stack(
                [_col_layout(ln1_g[l]) for l in range(n_layers)]
            )
            shared["ln1b"] = np.stack(
                [_col_layout(ln1_b[l]) for l in range(n_layers)]
            )
        if not ln_triv2:
            shared["ln2g"] = np.stack(
                [_col_layout(ln2_g[l]) for l in range(n_layers)]
            )
            shared["ln2b"] = np.stack(
                [_col_layout(ln2_b[l]) for l in range(n_layers)]
            )

    _F32_KEYS = {
        "bq_c_col", "bq_s_col", "bk_s_col", "b2c", "ln1g", "ln1b", "ln2g", "ln2b",
    }
    shared = {
        k: (v if k in _F32_KEYS else v.astype(bf_np)) for k, v in shared.items()
    }

    key = (
        n_layers, causal_self, self_needs_mask, cross_needs_mask, flags, tuple(TAPS),
    )
    if key not in _BUILD_CACHE:
        _BUILD_CACHE[key] = _build(
            n_layers, causal_self, self_needs_mask, cross_needs_mask, flags,
            tuple(TAPS),
        )
    nc = _BUILD_CACHE[key]

    pe = _pe_table()
    in_maps = []
    for b in range(B):
        m = dict(shared)
        m["x0T"] = _to_T_tiles(emb[ids[b]] + pe).astype(bf_np)
        m["encT"] = _to_T_tiles(enc[b]).astype(bf_np)
        if self_needs_mask:
            m["smaskT8"] = _mask_T8(smask[b]).astype(bf_np)
        if cross_needs_mask:
            m["cmaskT8"] = _mask_T8(cmask[b]).astype(bf_np)
        if not causal_self:
            m.pop("tri01", None)
        if not (self_needs_mask or cross_needs_mask):
            m.pop("ident", None)
        in_maps.append(m)

    res = run_bass_kernel_spmd(nc, in_maps, core_ids=list(range(8)))
    LAST_RESULT = res

    out = np.empty((B, S, D), np.float32)
    for b in range(B):
        xt = np.asarray(res.results[b]["out_xT"], np.float32)  # [P, NSUB, S]
        out[b] = xt.transpose(1, 0, 2).reshape(D, S).T
    return out
